# revision 1
# baseline (speedup 1.0000x reference)
"""BiMamba block Trainium2 kernel (8 NeuronCores, SPMD over 2 directions x 4 batches).

Self-contained: builds a Bass/Tile kernel at import-time constants, shards the
full inputs host-side (one (direction, batch) pair per core), runs via
run_bass_kernel_spmd, and recombines (final residual add on host in fp32).

Device-side pipeline per core (one direction, one sequence):
  LN stats -> normalize+transpose (PE) -> W_in projection (bf16 matmul) ->
  causal depthwise conv (DVE tap chain) -> silu -> Wx projection ->
  softplus(dt) via exp/ln -> per-(channel-block, state) selective scan with
  the native tensor_tensor_scan instruction (A = per-(d,s) ACT-exp scale) ->
  state contraction via PE identity-matmul PSUM accumulation -> gate ->
  output projection. All matmuls bf16 (fp32 PSUM accumulate), dt/dA fp32.
"""
import numpy as np
import ml_dtypes
from contextlib import ExitStack

import concourse.bacc as bacc
import concourse.bass as bass
import concourse.tile as tile
from concourse import mybir
from concourse.masks import make_identity
from concourse.bass_utils import run_bass_kernel_spmd

F32 = mybir.dt.float32
BF16 = mybir.dt.bfloat16
AF = mybir.ActivationFunctionType
OP = mybir.AluOpType
BF = ml_dtypes.bfloat16

D_MODEL = 768
D_INNER = 1536
D_STATE = 16
D_CONV = 4
DT_RANK = 48
L = 2048
B = 4


def _bcast_ap(dram_t, row, ncols, parts=128):
    src = dram_t[row:row + 1, 0:ncols]
    return bass.AP(tensor=src.tensor, offset=src.offset,
                   ap=[[0, parts]] + [list(src.ap[-1])])


def _build(L=L, DM=D_MODEL, DI=D_INNER, num_devices=8, eps=1e-5):
    NB = DI // 128
    KB = DM // 128
    FB = 2 * DI // 128
    NT = L // 128
    TC = L // 512
    NS = D_STATE
    PAD = D_CONV - 1

    nc = bacc.Bacc("TRN2", target_bir_lowering=False, debug=False,
                   enable_asserts=True, num_devices=num_devices)

    x_t = nc.dram_tensor("x_t", [L, DM], F32, kind="ExternalInput")
    w1t = nc.dram_tensor("w1t", [DM, 2 * DI], BF16, kind="ExternalInput")
    convw_r = nc.dram_tensor("convw_r", [128, NB * D_CONV], F32, kind="ExternalInput")
    ebx_r = nc.dram_tensor("ebx_r", [128, NB], F32, kind="ExternalInput")
    wxt = nc.dram_tensor("wxt", [DI, DT_RANK + 2 * NS], BF16, kind="ExternalInput")
    wdtt = nc.dram_tensor("wdtt", [DT_RANK, DI], BF16, kind="ExternalInput")
    wot = nc.dram_tensor("wot", [DI, DM], BF16, kind="ExternalInput")
    aexp_r = nc.dram_tensor("aexp_r", [128, NB * NS], F32, kind="ExternalInput")
    bdt_r = nc.dram_tensor("bdt_r", [128, NB], F32, kind="ExternalInput")
    dvec_r = nc.dram_tensor("dvec_r", [128, NB], F32, kind="ExternalInput")
    sbias_r = nc.dram_tensor("sbias_r", [128, NB], F32, kind="ExternalInput")
    ebz_r = nc.dram_tensor("ebz_r", [128, NB], F32, kind="ExternalInput")
    yout = nc.dram_tensor("yout", [DM, L], F32, kind="ExternalOutput")
    bc_dram = nc.dram_tensor("bc_dram", [2 * NS, L], BF16)

    with tile.TileContext(nc) as tc, ExitStack() as ctx:
        persist = ctx.enter_context(tc.tile_pool(name="persist", bufs=1))
        midp = ctx.enter_context(tc.tile_pool(name="midp", bufs=1))

        ident = persist.tile([128, 128], BF16)
        make_identity(nc, ident)
        eps_t = persist.tile([128, 1], F32)
        nc.vector.memset(eps_t, eps)
        aexp_sb = persist.tile([128, NB * NS], F32)
        nc.sync.dma_start(out=aexp_sb, in_=aexp_r.ap())
        bdt_sb = persist.tile([128, NB], F32)
        nc.sync.dma_start(out=bdt_sb, in_=bdt_r.ap())
        dvec_sb = persist.tile([128, NB], F32)
        nc.sync.dma_start(out=dvec_sb, in_=dvec_r.ap())
        sbias_sb = persist.tile([128, NB], F32)
        nc.sync.dma_start(out=sbias_sb, in_=sbias_r.ap())
        ebz_sb = persist.tile([128, NB], F32)
        nc.sync.dma_start(out=ebz_sb, in_=ebz_r.ap())
        ebx_sb = persist.tile([128, NB], F32)
        nc.sync.dma_start(out=ebx_sb, in_=ebx_r.ap())
        convw_sb = persist.tile([128, NB * D_CONV], F32)
        nc.sync.dma_start(out=convw_sb, in_=convw_r.ap())
        wdtt_sb = persist.tile([DT_RANK, DI], BF16)
        nc.sync.dma_start(out=wdtt_sb, in_=wdtt.ap())

        xc_sb = persist.tile([128, NB, L], BF16)
        xdbl48_sb = persist.tile([DT_RANK, L], BF16)
        bc_sb = persist.tile([2 * NS, L], BF16)
        z_sb = midp.tile([128, NB, L], BF16)

        # ---- phase 0: LN + transpose (xnt left-padded with PAD zero cols) ----
        with tc.tile_pool(name="ph01", bufs=1) as ph01, \
             tc.tile_pool(name="lnp", bufs=6) as lnp, \
             tc.tile_pool(name="tpsum", bufs=3, space="PSUM") as tpsum, \
             tc.tile_pool(name="mpsum", bufs=3, space="PSUM") as mpsum, \
             tc.tile_pool(name="convp", bufs=3) as convp:
            xnt = ph01.tile([128, KB, PAD + L], BF16)
            nc.vector.memset(xnt[:, :, 0:PAD], 0.0)
            mu_all = ph01.tile([128, NT], F32)
            r_all = ph01.tile([128, NT], F32)
            for tt in range(NT):
                x_tile = lnp.tile([128, DM], F32, tag="x_tile")
                nc.sync.dma_start(out=x_tile, in_=x_t[tt * 128:(tt + 1) * 128, :])
                nsub = DM // 256
                stats = lnp.tile([128, nsub, 6], F32, tag="stats")
                for i in range(nsub):
                    nc.vector.bn_stats(out=stats[:, i, :], in_=x_tile[:, i * 256:(i + 1) * 256])
                mv = lnp.tile([128, 2], F32, tag="mv")
                nc.vector.bn_aggr(out=mv, in_=stats)
                nc.vector.tensor_copy(out=mu_all[:, tt:tt + 1], in_=mv[:, 0:1])
                sq = lnp.tile([128, 1], F32, tag="sq")
                nc.scalar.activation(out=sq, in_=mv[:, 1:2], func=AF.Sqrt, bias=eps_t, scale=1.0)
                nc.vector.reciprocal(out=r_all[:, tt:tt + 1], in_=sq)
            for tt in range(NT):
                x_tile = lnp.tile([128, DM], F32, tag="x_tile")
                nc.sync.dma_start(out=x_tile, in_=x_t[tt * 128:(tt + 1) * 128, :])
                xn_bf = lnp.tile([128, DM], BF16, tag="xn_bf")
                nc.vector.tensor_scalar(out=xn_bf, in0=x_tile, scalar1=mu_all[:, tt:tt + 1],
                                        scalar2=r_all[:, tt:tt + 1], op0=OP.subtract, op1=OP.mult)
                for kb in range(KB):
                    tp = tpsum.tile([128, 128], BF16, tag="tp")
                    nc.tensor.transpose(tp, xn_bf[:, kb * 128:(kb + 1) * 128], ident)
                    nc.scalar.copy(out=xnt[:, kb, PAD + tt * 128: PAD + (tt + 1) * 128], in_=tp)

            # ---- phase 1: W1 projection + depthwise causal conv (DVE) + silu ----
            for fb in range(FB):
                is_x = fb < NB
                w1f = convp.tile([128, KB, 128], BF16, tag="w1f", bufs=3)
                for kb in range(KB):
                    nc.sync.dma_start(out=w1f[:, kb, :],
                                      in_=w1t[kb * 128:(kb + 1) * 128,
                                              fb * 128:(fb + 1) * 128])
                if is_x:
                    xzx = convp.tile([128, PAD + L], BF16, tag="xzx", bufs=3)
                    nc.vector.memset(xzx[:, 0:PAD], 0.0)
                for tcc in range(TC):
                    ps = mpsum.tile([128, 512], F32, tag="w1ps")
                    for kb in range(KB):
                        nc.tensor.matmul(ps, lhsT=w1f[:, kb, :],
                                         rhs=xnt[:, kb, PAD + tcc * 512: PAD + (tcc + 1) * 512],
                                         start=(kb == 0), stop=(kb == KB - 1))
                    if is_x:
                        nc.scalar.activation(
                            out=xzx[:, PAD + tcc * 512: PAD + (tcc + 1) * 512],
                            in_=ps, func=AF.Identity, bias=ebx_sb[:, fb:fb + 1], scale=1.0)
                    else:
                        zb = fb - NB
                        nc.scalar.activation(
                            out=z_sb[:, zb, tcc * 512:(tcc + 1) * 512],
                            in_=ps, func=AF.Silu, bias=ebz_sb[:, zb:zb + 1], scale=1.0)
                if is_x:
                    q0 = convp.tile([128, L], BF16, tag="q0", bufs=2)
                    q1 = convp.tile([128, L], BF16, tag="q1", bufs=2)
                    nc.vector.tensor_scalar(out=q0, in0=xzx[:, 0:L],
                                            scalar1=convw_sb[:, fb * D_CONV: fb * D_CONV + 1],
                                            scalar2=None, op0=OP.mult)
                    for k in range(1, D_CONV):
                        nc.vector.tensor_scalar(out=q1, in0=xzx[:, k:k + L],
                                                scalar1=convw_sb[:, fb * D_CONV + k:
                                                                 fb * D_CONV + k + 1],
                                                scalar2=None, op0=OP.mult)
                        nc.vector.tensor_tensor(out=q0, in0=q0, in1=q1, op=OP.add)
                    nc.scalar.activation(out=xc_sb[:, fb, :], in_=q0, func=AF.Silu,
                                         bias=sbias_sb[:, fb:fb + 1], scale=1.0)

            # ---- phase 3: Wx projection -> (B,C) rows + dt_rank rows ----
            with tc.tile_pool(name="wxp", bufs=1) as wxp, \
                 tc.tile_pool(name="xpsum", bufs=2, space="PSUM") as xpsum:
                wxt_sb = wxp.tile([128, NB, DT_RANK + 2 * NS], BF16)
                for kb in range(NB):
                    nc.sync.dma_start(out=wxt_sb[:, kb, :], in_=wxt[kb * 128:(kb + 1) * 128, :])
                for tcc in range(TC):
                    ps = xpsum.tile([DT_RANK + 2 * NS, 512], F32, tag="wxps")
                    for kb in range(NB):
                        nc.tensor.matmul(ps, lhsT=wxt_sb[:, kb, :],
                                         rhs=xc_sb[:, kb, tcc * 512:(tcc + 1) * 512],
                                         start=(kb == 0), stop=(kb == NB - 1))
                    nc.scalar.copy(out=bc_sb[:, tcc * 512:(tcc + 1) * 512], in_=ps[0:2 * NS, :])
                    nc.scalar.copy(out=xdbl48_sb[0:32, tcc * 512:(tcc + 1) * 512],
                                   in_=ps[2 * NS:2 * NS + 32, :])
                    nc.scalar.copy(out=xdbl48_sb[32:DT_RANK, tcc * 512:(tcc + 1) * 512],
                                   in_=ps[2 * NS + 32:2 * NS + DT_RANK, :])
                nc.sync.dma_start(out=bc_dram.ap(), in_=bc_sb)

        # ---- scan phase ----
        with tc.tile_pool(name="scanp", bufs=1) as sp1, \
             tc.tile_pool(name="scan2", bufs=2) as sp2, \
             tc.tile_pool(name="ypsum", bufs=1, space="PSUM") as ypsum, \
             tc.tile_pool(name="dtpsum", bufs=3, space="PSUM") as dtpsum:
            for db in range(NB):
                dt_sb = sp1.tile([128, L], F32, tag="dt")
                for tcc in range(TC):
                    dps = dtpsum.tile([128, 512], F32, tag="dtps")
                    nc.tensor.matmul(dps, lhsT=wdtt_sb[:, db * 128:(db + 1) * 128],
                                     rhs=xdbl48_sb[:, tcc * 512:(tcc + 1) * 512],
                                     start=True, stop=True)
                    edt = sp2.tile([128, 512], F32, tag="edt")
                    nc.scalar.activation(out=edt, in_=dps, func=AF.Exp,
                                         bias=bdt_sb[:, db:db + 1], scale=1.0)
                    nc.scalar.activation(out=dt_sb[:, tcc * 512:(tcc + 1) * 512], in_=edt,
                                         func=AF.Ln, bias=1.0, scale=1.0)
                ux = sp1.tile([128, L], BF16, tag="ux")
                nc.vector.tensor_tensor(out=ux, in0=dt_sb, in1=xc_sb[:, db, :], op=OP.mult)
                y_ps = ypsum.tile([128, L], F32, tag="yps")
                for s in range(D_STATE):
                    bbc = sp2.tile([128, L], BF16, tag="bbc")
                    nc.sync.dma_start(out=bbc, in_=_bcast_ap(bc_dram, s, L))
                    cbc = sp2.tile([128, L], BF16, tag="cbc")
                    nc.sync.dma_start(out=cbc, in_=_bcast_ap(bc_dram, D_STATE + s, L))
                    dA = sp2.tile([128, L], F32, tag="dA", bufs=3)
                    nc.scalar.activation(out=dA, in_=dt_sb, func=AF.Exp,
                                         scale=aexp_sb[:, db * D_STATE + s: db * D_STATE + s + 1])
                    up = sp1.tile([128, L], BF16, tag="up", bufs=2)
                    nc.vector.tensor_tensor(out=up, in0=ux, in1=bbc, op=OP.mult)
                    h = sp1.tile([128, L], BF16, tag="h", bufs=2)
                    nc.vector.tensor_tensor_scan(out=h, data0=dA, data1=up, initial=0.0,
                                                 op0=OP.mult, op1=OP.add)
                    yc = sp2.tile([128, L], BF16, tag="yc", bufs=3)
                    nc.vector.tensor_tensor(out=yc, in0=h, in1=cbc, op=OP.mult)
                    for c in range(TC):
                        nc.tensor.matmul(y_ps[:, c * 512:(c + 1) * 512], lhsT=ident,
                                         rhs=yc[:, c * 512:(c + 1) * 512],
                                         start=(s == 0), stop=(s == D_STATE - 1))
                t1 = sp1.tile([128, L], BF16, tag="t1", bufs=2)
                nc.vector.scalar_tensor_tensor(out=t1, in0=xc_sb[:, db, :],
                                               scalar=dvec_sb[:, db:db + 1], in1=y_ps,
                                               op0=OP.mult, op1=OP.add)
                nc.vector.tensor_tensor(out=xc_sb[:, db, :], in0=t1, in1=z_sb[:, db, :],
                                        op=OP.mult)

        # ---- Wout ----
        with tc.tile_pool(name="woutp", bufs=1) as wop, \
             tc.tile_pool(name="wevac", bufs=3) as wevac, \
             tc.tile_pool(name="opsum", bufs=3, space="PSUM") as opsum:
            wot_sb = wop.tile([128, NB, DM], BF16)
            for kb in range(NB):
                nc.sync.dma_start(out=wot_sb[:, kb, :], in_=wot[kb * 128:(kb + 1) * 128, :])
            for ob in range(DM // 128):
                for tcc in range(TC):
                    ps = opsum.tile([128, 512], F32, tag="ops")
                    for kb in range(NB):
                        nc.tensor.matmul(ps, lhsT=wot_sb[:, kb, ob * 128:(ob + 1) * 128],
                                         rhs=xc_sb[:, kb, tcc * 512:(tcc + 1) * 512],
                                         start=(kb == 0), stop=(kb == NB - 1))
                    yo = wevac.tile([128, 512], F32, tag="yo")
                    nc.scalar.copy(out=yo, in_=ps)
                    nc.sync.dma_start(out=yout[ob * 128:(ob + 1) * 128,
                                               tcc * 512:(tcc + 1) * 512], in_=yo)
    nc.compile()
    return nc


def _reshape_r(v, nblk):
    return np.ascontiguousarray(v.reshape(nblk, 128).T)


def _prep_core_inputs(inputs, direction, b):
    di = D_INNER
    nblk = di // 128
    p = direction
    W_in = np.asarray(inputs[p + '_Win'], np.float32)
    g = np.asarray(inputs['ln_g'], np.float32)
    lb = np.asarray(inputs['ln_b'], np.float32)
    w1 = W_in * g[None, :]
    c0 = W_in @ lb
    convw = np.asarray(inputs[p + '_convw'], np.float32)
    convw_r = np.ascontiguousarray(
        convw.reshape(nblk, 128, D_CONV).transpose(1, 0, 2).reshape(128, nblk * D_CONV))
    A = -np.exp(np.asarray(inputs[p + '_Alog'], np.float32))
    aexp_r = np.ascontiguousarray(
        A.reshape(nblk, 128, D_STATE).transpose(1, 0, 2).reshape(128, nblk * D_STATE))
    Wx = np.asarray(inputs[p + '_Wx'], np.float32)
    x = np.asarray(inputs['x'], np.float32)[b]
    if direction == 'b':
        x = x[::-1]
    return {
        'x_t': np.ascontiguousarray(x),
        'w1t': np.ascontiguousarray(w1.T).astype(BF),
        'convw_r': convw_r,
        'wxt': np.ascontiguousarray(
            np.concatenate([Wx[DT_RANK:], Wx[:DT_RANK]], 0).T).astype(BF),
        'wdtt': np.ascontiguousarray(np.asarray(inputs[p + '_Wdt'], np.float32).T).astype(BF),
        'wot': np.ascontiguousarray(np.asarray(inputs[p + '_Wout'], np.float32).T).astype(BF),
        'aexp_r': aexp_r,
        'bdt_r': _reshape_r(np.asarray(inputs[p + '_bdt'], np.float32), nblk),
        'dvec_r': _reshape_r(np.asarray(inputs[p + '_D'], np.float32), nblk),
        'sbias_r': _reshape_r(np.asarray(inputs[p + '_convb'], np.float32), nblk),
        'ebx_r': _reshape_r(c0[:di], nblk),
        'ebz_r': _reshape_r(c0[di:], nblk),
    }


_NC = None


def _get_nc():
    global _NC
    if _NC is None:
        _NC = _build()
    return _NC


def kernel(**inputs) -> np.ndarray:
    nc = _get_nc()
    in_maps = []
    for c in range(8):
        d = 'f' if c < 4 else 'b'
        in_maps.append(_prep_core_inputs(inputs, d, c % 4))
    res = run_bass_kernel_spmd(nc, in_maps, list(range(8)), trace=False)
    x = np.asarray(inputs['x'], np.float32)
    out = x.copy()
    for b in range(B):
        out[b] += res.results[b]["yout"].T
        out[b] += res.results[4 + b]["yout"].T[::-1]
    return out

